# revision 34
# baseline (speedup 1.0000x reference)
"""Multi-head attention (B=2, S=2048, H=1024, NH=16) on 8 TRN2 NeuronCores.

Sharding: fully data/tensor parallel, no collectives. Core c = (b, hg) with
b = c // 4 (batch), hg = c % 4 (head group of 4 heads = 256 of the 1024
projection output dims).

v11 design (best measured: ~214us, vs 276us staged baseline):
  - No PE transposes: q/k projections produce qT/kT [256, S] (W stationary,
    xT moving); the v projection uses the opposite orientation (x chunk
    stationary, WvT moving) producing v directly in the [k, d] layout the
    context matmul needs; context goes out UNNORMALIZED as ctxT
    [65 rows/head, S] (row 64 = softmax denominator); host divides +
    transposes in fp32.
  - ACT (exp: 128 x ~1.09us = 140us) paces the attention phase. Scores
    rounds (h, pr, kc) = two same-row-group K=64 matmuls into a [128,1024]
    PSUM tile + one 1024-wide exp. Rounds are paced into the projection /
    v streams; the final phase interleaves 2-round scores bursts with ctx
    bursts (i-split 512-col accumulator streams, 2 PSUM banks) to minimize
    PE pipeline breaks.
  - DMA: weights host-repacked to the SBUF layout; pair-0 x-tile halves
    issue on both the sync and ACT sequencers (each DMA issue costs ~620ns
    of sequencer time, which otherwise starves the early projections);
    per-queue DMA is only ~22.5 GB/s so depth/parallelism matters.
  - The va zero/ones init runs after the pair-0 projections: putting it
    first stalls the PE on the psA WAR behind a ~13us DVE memset queue.
  - PSUM: proj 4 banks + scores ring 2x[128,1024]; final phase scores ring
    3 (psS 2 + psF 1) + ctx 2 banks.
"""

import functools
import sys

if "/opt/trn_rl_repo" not in sys.path:
    sys.path.insert(0, "/opt/trn_rl_repo")

import numpy as np

B, S, H = 2, 2048, 1024
NH, HD = 16, 64
NCORES = 8
GROUPS = 4                # head groups (cores per batch)
DPG = H // GROUPS         # projection dims per core = 256
HPG = DPG // HD           # heads per core = 4
P = 128                   # SBUF partitions
NHC = H // P              # contraction chunks per projection = 8
QB = 512                  # q block (matmul moving free dim)
NQB = S // QB             # 4
NKC = S // P              # k chunks = 16
VA_W = HD + 1             # 64 v dims + ones col (softmax denominator)
VA_PAD = 128              # va slot width (padded; FWL + zero pad rows)
NVG = 4                   # v projection groups (4 kc chunks each)

PACE = 1300               # PE cols consumed per scores round while filling
CTX_LAG = 3               # rounds of scores lead required before ctx reads


@functools.lru_cache(maxsize=1)
def _build():
    import concourse.bacc as bacc
    import concourse.mybir as mybir
    import concourse.tile as tile
    from collections import deque

    F32 = mybir.dt.float32
    BF16 = mybir.dt.bfloat16
    Exp = mybir.ActivationFunctionType.Exp
    ADD = mybir.AluOpType.add

    nc = bacc.Bacc()

    xq_d = nc.declare_dram_parameter("xq", [H, S], BF16, isOutput=False)
    xk_d = nc.declare_dram_parameter("xk", [H, S], BF16, isOutput=False)
    xv_d = nc.declare_dram_parameter("xv", [H, S], BF16, isOutput=False)
    # weights host-repacked to the exact SBUF layout [P, NHC*DPG]
    wq_d = nc.declare_dram_parameter("wq", [P, NHC * DPG], BF16, isOutput=False)
    wk_d = nc.declare_dram_parameter("wk", [P, NHC * DPG], BF16, isOutput=False)
    wv_d = nc.declare_dram_parameter("wv", [P, NHC * DPG], BF16, isOutput=False)
    bqk_d = nc.declare_dram_parameter("bqk", [P, 4], F32, isOutput=False)
    bvb_d = nc.declare_dram_parameter("bvb", [P, DPG], F32, isOutput=False)
    mk_d = nc.declare_dram_parameter("mk", [P, NKC], F32, isOutput=False)
    out_d = nc.declare_dram_parameter("out", [HPG * VA_W, S], F32, isOutput=True)

    # scores-round emission order (h, pr, kc); pr = pair of q blocks
    rounds = (
        [(h, 0, kc) for h in range(HPG) for kc in range(8)]          # rA
        + [(h, 0, kc) for h in range(HPG) for kc in range(8, 16)]    # rB
        + [(h, 1, kc) for h in range(HPG) for kc in range(16)]       # rC
    )
    NR = len(rounds)  # 128
    ridx = {hpk: r for r, hpk in enumerate(rounds)}
    pq = [None] * NR
    pq_reads = [0] * NR

    # ctx consumption order: per (h, pr), the two i accumulator streams
    # interleave kc-by-kc (both fit in PSUM: tags A/B) so the last pair
    # doesn't serialize 16 units after the final exp
    ctx_units = [
        (h, pr, i, kc)
        for pr in range(2)
        for h in range(HPG)
        for kc in range(NKC)
        for i in range(2)
    ]

    with tile.TileContext(nc) as tc:
        with (
            tc.tile_pool(name="const", bufs=1) as cpool,
            tc.tile_pool(name="proj_out", bufs=1) as projpool,
            tc.tile_pool(name="xt", bufs=14) as xpool,
            tc.tile_pool(name="xvp", bufs=12) as xvpool,
            tc.tile_pool(name="pexp", bufs=48) as ppool,
            tc.tile_pool(name="outb", bufs=4) as opool,
        ):
            wk_sb = cpool.tile([P, NHC * DPG], BF16)
            wq_sb = cpool.tile([P, NHC * DPG], BF16)
            wv_sb = cpool.tile([P, NHC * DPG], BF16)
            bqk_sb = cpool.tile([P, 4], F32)
            bvb_sb = cpool.tile([P, DPG], F32)
            mk_sb = cpool.tile([P, NKC], F32)

            qT0 = projpool.tile([P, S], BF16)
            qT1 = projpool.tile([P, S], BF16)
            kT0 = projpool.tile([P, S], BF16)
            kT1 = projpool.tile([P, S], BF16)
            va_sb = projpool.tile([P, NKC * HPG, VA_PAD], BF16)

            # weight/const issues via the ACT sequencer (idle until first
            # exp); the first wk stripe gates the first matmul
            WSTR = (NHC * DPG) // 4  # stripe cols (2 hc chunks) = 512
            for s in range(2):
                nc.scalar.dma_start(
                    wk_sb[:, s * WSTR : (s + 1) * WSTR],
                    wk_d[:, s * WSTR : (s + 1) * WSTR],
                )

            # ---- scores round machinery ----
            state = {"emitted": 0, "acc": 0, "ready": 0}

            def scores_round(r, pool=None, sbufs=2):
                h, pr, kc = rounds[r]
                qT_t = qT0 if h < 2 else qT1
                kT_t = kT0 if h < 2 else kT1
                rows = slice((h % 2) * HD, (h % 2) * HD + HD)
                p2 = ppool.tile([P, 2 * QB], BF16, tag="p", name=f"p{r}")
                s2 = (pool or psS).tile(
                    [P, 2 * QB], F32, tag="s2", name=f"s2_{r}", bufs=sbufs
                )
                for i in range(2):
                    qb = pr * 2 + i
                    nc.tensor.matmul(
                        s2[:, i * QB : (i + 1) * QB],
                        kT_t[rows, kc * P : (kc + 1) * P],
                        qT_t[rows, qb * QB : (qb + 1) * QB],
                        start=True,
                        stop=True,
                    )
                nc.scalar.activation(
                    p2[:], s2[:], Exp, bias=mk_sb[:, kc : kc + 1], scale=0.125
                )
                pq[r] = p2

            def pump(cols):
                state["acc"] += cols
                while state["acc"] >= PACE and state["emitted"] < state["ready"]:
                    scores_round(state["emitted"])
                    state["emitted"] += 1
                    state["acc"] = max(state["acc"] - PACE, 0)

            # ---- q/k projections (W stationary, xT moving); x-tile DMA
            # halves alternate between the sync and ACT sequencers ----
            def proj_pair(x_d, w_sb, bcol, dst0, dst1, pr, psA, fill,
                          extra_dma=None, dual=False):
                cols0 = pr * 2 * QB
                pp = [
                    psA.tile([P, QB], F32, tag=f"pp{j}", name=f"pp{j}", bufs=1)
                    for j in range(4)
                ]
                for hc in range(NHC):
                    if extra_dma is not None:
                        extra_dma(hc)
                    xt = xpool.tile([P, 2 * QB], BF16, tag="xt", name="xt")
                    engs = ((0, nc.sync), (1, nc.scalar if dual else nc.sync))
                    for sp, eng in engs:
                        eng.dma_start(
                            xt[:, sp * QB : (sp + 1) * QB],
                            x_d[
                                hc * P : (hc + 1) * P,
                                cols0 + sp * QB : cols0 + (sp + 1) * QB,
                            ],
                        )
                    st = dict(start=(hc == 0), stop=(hc == NHC - 1))
                    w0 = w_sb[:, hc * DPG : hc * DPG + P]
                    w1 = w_sb[:, hc * DPG + P : (hc + 1) * DPG]
                    nc.tensor.matmul(pp[0][:], w0, xt[:, :QB], **st)
                    nc.tensor.matmul(pp[1][:], w0, xt[:, QB:], **st)
                    nc.tensor.matmul(pp[2][:], w1, xt[:, :QB], **st)
                    nc.tensor.matmul(pp[3][:], w1, xt[:, QB:], **st)
                    if fill:
                        pump(4 * QB)
                for j in range(4):
                    dst = dst0 if j < 2 else dst1
                    bc = bcol + (0 if j < 2 else 1)
                    qb = pr * 2 + (j % 2)
                    nc.vector.tensor_scalar(
                        dst[:, qb * QB : (qb + 1) * QB], pp[j][:],
                        bqk_sb[:, bc : bc + 1], None, ADD,
                    )

            with tc.tile_pool(name="psS", bufs=1, space="PSUM") as psS:
                with tc.tile_pool(name="psA", bufs=1, space="PSUM") as psA:
                    def dma_wq(hc):
                        if hc in (1, 2):
                            s = hc + 1
                            nc.scalar.dma_start(
                                wk_sb[:, s * WSTR : (s + 1) * WSTR],
                                wk_d[:, s * WSTR : (s + 1) * WSTR],
                            )
                        if hc >= 4:
                            s = hc - 4
                            nc.scalar.dma_start(
                                wq_sb[:, s * WSTR : (s + 1) * WSTR],
                                wq_d[:, s * WSTR : (s + 1) * WSTR],
                            )
                        if hc == 7:
                            nc.scalar.dma_start(bqk_sb[:], bqk_d[:])
                            nc.scalar.dma_start(mk_sb[:], mk_d[:])

                    def dma_wv(hc):
                        if hc == 0:
                            nc.scalar.dma_start(bvb_sb[:], bvb_d[:])
                        if hc % 2 == 0:
                            s = hc // 2
                            nc.scalar.dma_start(
                                wv_sb[:, s * WSTR : (s + 1) * WSTR],
                                wv_d[:, s * WSTR : (s + 1) * WSTR],
                            )

                    proj_pair(xk_d, wk_sb, 2, kT0, kT1, 0, psA, False,
                              extra_dma=dma_wq, dual=True)
                    proj_pair(xq_d, wq_sb, 0, qT0, qT1, 0, psA, False,
                              extra_dma=dma_wv, dual=True)
                    state["ready"] = 32
                    # va zero pad + ones cols: DVE is idle from here until
                    # the v phase; doing this earlier delays the projection
                    # bias reads and stalls the PE on the psA WAR
                    nc.vector.memset(va_sb[:], 0.0)
                    nc.vector.memset(va_sb[:, :, HD : HD + 1], 1.0)
                    proj_pair(xk_d, wk_sb, 2, kT0, kT1, 1, psA, True)
                    state["ready"] = 64
                    proj_pair(xq_d, wq_sb, 0, qT0, qT1, 1, psA, True)
                    state["ready"] = NR

                # ---- v projection (x stationary, WvT moving), 4-kc groups ----
                with tc.tile_pool(name="psV", bufs=1, space="PSUM") as psV:
                    for g in range(NVG):
                        cols0 = g * 4 * P
                        xvt = []
                        for hc in range(NHC):
                            xt = xvpool.tile(
                                [P, 4 * P], BF16, tag="xv", name=f"xv{g}_{hc}"
                            )
                            nc.sync.dma_start(
                                xt[:],
                                xv_d[hc * P : (hc + 1) * P, cols0 : cols0 + 4 * P],
                            )
                            xvt.append(xt)
                        vp = [
                            psV.tile([P, DPG], F32, tag=f"vp{i}",
                                     name=f"vp{i}", bufs=1)
                            for i in range(4)
                        ]
                        for hc in range(NHC):
                            st = dict(start=(hc == 0), stop=(hc == NHC - 1))
                            for i in range(4):
                                nc.tensor.matmul(
                                    vp[i][:],
                                    xvt[hc][:, i * P : (i + 1) * P],
                                    wv_sb[:, hc * DPG : (hc + 1) * DPG],
                                    **st,
                                )
                            pump(4 * DPG)
                        for i in range(4):
                            kc = g * 4 + i
                            for h in range(HPG):
                                nc.vector.tensor_tensor(
                                    va_sb[:, kc * HPG + h, :HD],
                                    vp[i][:, h * HD : (h + 1) * HD],
                                    bvb_sb[:, h * HD : (h + 1) * HD],
                                    ADD,
                                )

                # ---- final phase: 2-round scores bursts + ctx bursts ----
                with (
                    tc.tile_pool(name="psC", bufs=1, space="PSUM") as psC,
                    tc.tile_pool(name="psF", bufs=1, space="PSUM") as psF,
                ):
                    cq = deque(ctx_units)
                    cur = {}

                    def ctx_unit():
                        h, pr, i, kc = cq.popleft()
                        r = ridx[(h, pr, kc)]
                        key = (h, pr, i)
                        if key not in cur:
                            par = "A" if (pr * HPG * 2 + h * 2 + i) % 2 == 0 else "B"
                            cur[key] = psC.tile(
                                [VA_PAD, QB], F32, tag=f"ct{par}",
                                name=f"ct{h}_{pr}_{i}", bufs=1,
                            )
                        ct = cur[key]
                        nc.tensor.matmul(
                            ct[:],
                            va_sb[:, kc * HPG + h, :],
                            pq[r][:, i * QB : (i + 1) * QB],
                            start=(kc == 0),
                            stop=(kc == NKC - 1),
                        )
                        pq_reads[r] += 1
                        if pq_reads[r] == 2:
                            pq[r] = None
                        if kc == NKC - 1:
                            ct = cur.pop(key)
                            qb = pr * 2 + i
                            ob = opool.tile(
                                [VA_W, QB], F32, tag="ob",
                                name=f"ob{h}_{pr}_{i}",
                            )
                            nc.vector.tensor_copy(ob[:], ct[:VA_W, :])
                            for sp in range(2):
                                c0 = qb * QB + sp * (QB // 2)
                                nc.sync.dma_start(
                                    out_d[
                                        h * VA_W : (h + 1) * VA_W,
                                        c0 : c0 + QB // 2,
                                    ],
                                    ob[:, sp * (QB // 2) : (sp + 1) * (QB // 2)],
                                )

                    # per round: scores + col-budgeted ctx catch-up
                    nf = 0
                    fill = 0
                    for r in range(state["emitted"], NR):
                        nf += 1
                        if nf % 3 == 0:
                            scores_round(r, psF, sbufs=1)
                        else:
                            scores_round(r)
                        fill += 1467
                        while cq and fill >= QB:
                            h2, pr2, i2, kc2 = cq[0]
                            if ridx[(h2, pr2, kc2)] + CTX_LAG <= r + 1:
                                ctx_unit()
                                fill -= QB
                            else:
                                break
                    while cq:
                        ctx_unit()

    nc.compile()
    return nc


def _in_maps(query, key, value, attention_mask, Wq, bq, Wk, bk, Wv, bv):
    import ml_dtypes

    bf16 = ml_dtypes.bfloat16
    q = np.asarray(query, np.float32)
    k = np.asarray(key, np.float32)
    v = np.asarray(value, np.float32)
    m = np.asarray(attention_mask, np.float32)
    Wq = np.asarray(Wq, np.float32)
    Wk = np.asarray(Wk, np.float32)
    Wv = np.asarray(Wv, np.float32)
    bq = np.asarray(bq, np.float32)
    bk = np.asarray(bk, np.float32)
    bv = np.asarray(bv, np.float32)

    def repack_w(W, hs, he):
        wt = np.ascontiguousarray(W[hs:he, :].T)        # [H, DPG]
        wr = wt.reshape(NHC, P, DPG).transpose(1, 0, 2).reshape(P, NHC * DPG)
        return np.ascontiguousarray(wr).astype(bf16)

    xT = [
        (
            np.ascontiguousarray(q[b].T).astype(bf16),
            np.ascontiguousarray(k[b].T).astype(bf16),
            np.ascontiguousarray(v[b].T).astype(bf16),
        )
        for b in range(B)
    ]
    maps = []
    for c in range(NCORES):
        b, hg = divmod(c, GROUPS)
        hs = hg * DPG
        he = hs + DPG
        bqs, bks = bq[hs:he], bk[hs:he]
        bqk = np.stack([bqs[:P], bqs[P:], bks[:P], bks[P:]], axis=1).astype(
            np.float32
        )
        bvb = np.ascontiguousarray(
            np.broadcast_to(bv[hs:he][None, :], (P, DPG)).astype(np.float32)
        )
        maps.append(
            {
                "xq": xT[b][0],
                "xk": xT[b][1],
                "xv": xT[b][2],
                "wq": repack_w(Wq, hs, he),
                "wk": repack_w(Wk, hs, he),
                "wv": repack_w(Wv, hs, he),
                "bqk": np.ascontiguousarray(bqk),
                "bvb": bvb,
                "mk": np.ascontiguousarray(m[b, 0, 0].reshape(NKC, P).T),
            }
        )
    return maps


def kernel(query, key, value, attention_mask, Wq, bq, Wk, bk, Wv, bv):
    from concourse.bass_utils import run_bass_kernel_spmd

    nc = _build()
    maps = _in_maps(
        query, key, value, attention_mask, Wq, bq, Wk, bk, Wv, bv
    )
    res = run_bass_kernel_spmd(nc, maps, core_ids=list(range(NCORES)))
    out = np.empty((B, S, H), np.float32)
    for c in range(NCORES):
        b, hg = divmod(c, GROUPS)
        o = np.asarray(res.results[c]["out"], np.float32)  # [4*65, S]
        for h in range(HPG):
            blk = o[h * VA_W : (h + 1) * VA_W]
            ctx = blk[:HD] / blk[HD : HD + 1]
            out[b, :, hg * DPG + h * HD : hg * DPG + (h + 1) * HD] = ctx.T
    return out
